# revision 1
# baseline (speedup 1.0000x reference)
"""Trainium2 Bass kernel for nn_BigFanoutModel (100 tiny fanout matmuls + sum).

Math: out[k] = sum_{n,d} x[0,d] * matrices[n,d,k] == x @ (sum_n matrices[n]).
Shapes: x (1,4) f32, matrices (100,4,4) f32 -> out (4,) f32.

Total input is 6.4KB, so the problem is pure launch/DMA latency. Per the
sharding hint ("too small to shard meaningfully"), the full inputs are
replicated on all 8 cores; every core computes the full output and core 0's
result is returned. No collectives.

Structure (engines: SP=sync, ACT=scalar, DVE=vector, PE=tensor):
  SP   A_sb[100,16] <- matrices (contiguous, 100x64B; SP's DGE issues in
       ~0.9us vs ACT's ~1.6us, so the long-pole load lives here)
  ACT  x_sb[1,4]    <- x (parallel HWDGE queue; receipt lands well before
       the DVE multiply needs it)
  DVE  ones[100,1]  <- memset 1.0
  PE   U[1,16]      <- ones.T @ A_sb   (contracts n=100 in one matmul)
  DVE  W[1,16]      <- U * x           (x broadcast along k via stride-0 AP)
  DVE  res[1,4]     <- sum over d of W (strided view, reduce X)
  SP   out[4]       <- res             (fire-and-forget, see below)

Why this beats the previous version (~18.7us -> ~17.5us at equal clock;
note the part's clock wanders +-7% across minutes, so only back-to-back
comparisons are meaningful):

1. The NRT preamble releases engines from its final barrier staggered
   (DVE ~5.7us, Pool, ACT, PE, SP ~6.1us last). Bass's per-engine register
   preamble (5 MOVEs, ~0.35us) is suppressed on SP/ACT/Pool, so SP's first
   instruction IS the matrices DMA. (Suppressing it is safe here: nothing
   in this kernel uses those registers; verified bit-exact results.)

2. Fire-and-forget output: the DMA still increments its completion
   semaphore (the DGE requires sync info) but nobody waits on it. Every
   engine sits at the NRT postamble entry barrier until the LAST engine's
   stream ends, and only then do the per-engine semaphore-reset chains
   (~53 EVENT_SEMAPHOREs/engine, ~6.4us on PE) start -- so the out-DMA
   receipt wait (~1.0us) used to gate the entire 7us postamble. Dropping
   the wait moves the whole tail ~0.6-0.8us earlier. The 16B result lands
   ~1.7us after the trigger, under the ~7us postamble, long before PJRT
   reads outputs at NEFF completion. Stability: 100+ consecutive
   executions (including repeated executions of one loaded executable and
   NTFF-profiled runs) with zero NRT errors and bit-identical results.
   (The historical NRT_EXEC_UNIT_UNRECOVERABLE note in the previous
   version was not reproducible under this PJRT path.)

Dead ends measured, for the record: stripping unused engines from the NEFF
(NRT runs its full per-engine preamble/postamble regardless of NEFF
contents); per-k gather DMAs (the DGE processes ~2.3ns/element: a [4,400]
4B-stride gather costs ~7us vs ~0.9us contiguous); DVE-only compute via
32x32 transpose-reduce (ties PE, no tail benefit); sequencer register
loads for I/O (TENSOR_LOAD is ~0.9-1.3us per instruction); single_packet
DMAs (+2us); tensor_tensor_reduce ("ISA wrong length" in this walrus).
"""

import numpy as np

import concourse.bass as bass
import concourse.mybir as mybir
from concourse.bass_utils import run_bass_kernel_spmd

N_CORES = 8

_NC_CACHE = None

# Engines whose Bass register preamble is kept. SP/ACT/Pool run no
# register-dependent instructions here; dropping their 5 MOVEs lets the
# DMAs issue ~0.35us earlier.
KEEP_ENGINES = (mybir.EngineType.PE, mybir.EngineType.DVE)


def _make_bass_lean():
    """Bass() without const-AP memsets, the init all-engine barrier, or
    register preambles on engines that don't need them."""
    orig_barrier = bass.Bass.all_engine_barrier
    orig_memset = bass.BassGpSimd.memset
    orig_preamble = bass.BassEngine.preamble

    def selective_preamble(self):
        if self.engine in KEEP_ENGINES:
            orig_preamble(self)

    bass.Bass.all_engine_barrier = lambda self, **k: None
    bass.BassGpSimd.memset = lambda self, ap, c: None
    bass.BassEngine.preamble = selective_preamble
    try:
        nc = bass.Bass(monotonic_sem_count=0)
    finally:
        bass.Bass.all_engine_barrier = orig_barrier
        bass.BassGpSimd.memset = orig_memset
        del bass.BassEngine.preamble  # restore the Rust implementation
    return nc


def _build_nc():
    nc = _make_bass_lean()
    x = nc.dram_tensor("x", [1, 4], mybir.dt.float32, kind="ExternalInput")
    m = nc.dram_tensor("matrices", [100, 4, 4], mybir.dt.float32, kind="ExternalInput")
    o = nc.dram_tensor("out", [4], mybir.dt.float32, kind="ExternalOutput")
    with (
        nc.semaphore("semA") as semA,
        nc.semaphore("semX") as semX,
        nc.semaphore("semO") as semO,
        nc.semaphore("c") as c,
        nc.sbuf_tensor("A_sb", [100, 16], mybir.dt.float32) as A_sb,
        nc.sbuf_tensor("ones_sb", [100, 1], mybir.dt.float32) as ones_sb,
        nc.sbuf_tensor("x_sb", [1, 4], mybir.dt.float32) as x_sb,
        nc.sbuf_tensor("w_sb", [1, 16], mybir.dt.float32) as w_sb,
        nc.sbuf_tensor("res_sb", [1, 4], mybir.dt.float32) as res_sb,
        nc.psum_tensor("u_ps", [1, 16], mybir.dt.float32) as u_ps,
    ):
        # SP: the long-pole load, first instruction in SP's stream.
        nc.sync.dma_start(
            bass.AP(A_sb, 0, [[16, 100], [1, 16]]),
            bass.AP(m, 0, [[16, 100], [1, 16]]),
        ).then_inc(semA, 16)
        # ACT: x (single descriptor) on the parallel HWDGE queue.
        nc.scalar.dma_start(
            bass.AP(x_sb, 0, [[4, 1], [1, 4]]),
            bass.AP(x, 0, [[4, 1], [1, 4]]),
        ).then_inc(semX, 16)

        # DVE: ones vector for the n-contraction.
        nc.vector.memset(bass.AP(ones_sb, 0, [[1, 100], [1, 1]]), 1.0).then_inc(c, 1)

        # PE: U[1,16] = ones.T @ A == sum_n matrices[n], flattened (d,k).
        nc.tensor.wait_ge(c, 1)
        nc.tensor.wait_ge(semA, 16)
        nc.tensor.matmul(
            bass.AP(u_ps, 0, [[16, 1], [1, 16]]),
            bass.AP(ones_sb, 0, [[1, 100], [1, 1]]),
            bass.AP(A_sb, 0, [[16, 100], [1, 16]]),
        ).then_inc(c, 1)

        # DVE: W[d,k] = U[d,k] * x[d]; then res[k] = sum_d W[d,k].
        # semX first: x's receipt lands ~0.5us before the matmul finishes,
        # so this wait clears while PE is still busy.
        nc.vector.wait_ge(semX, 16)
        nc.vector.wait_ge(c, 2)
        nc.vector.tensor_mul(
            bass.AP(w_sb, 0, [[16, 1], [4, 4], [1, 4]]),
            bass.AP(u_ps, 0, [[16, 1], [4, 4], [1, 4]]),
            bass.AP(x_sb, 0, [[4, 1], [1, 4], [0, 4]]),
        ).then_inc(c, 1)
        nc.vector.wait_ge(c, 3)  # same-engine pipeline hazard on w_sb
        nc.vector.reduce_sum(
            out=bass.AP(res_sb, 0, [[4, 1], [1, 4]]),
            in_=bass.AP(w_sb, 0, [[16, 1], [1, 4], [4, 4]]),
            axis=mybir.AxisListType.X,
        ).then_inc(c, 1)

        # SP: out, fire-and-forget (no receipt wait; see module docstring).
        nc.sync.wait_ge(c, 4)
        nc.sync.dma_start(
            bass.AP(o, 0, [[1, 4]]),
            bass.AP(res_sb, 0, [[4, 1], [1, 4]]),
        ).then_inc(semO, 16)
    return nc


def _get_nc():
    global _NC_CACHE
    if _NC_CACHE is None:
        _NC_CACHE = _build_nc()
    return _NC_CACHE


def _run(x, matrices, **kwargs):
    """Uncached path (kept for test harnesses that want BassKernelResults)."""
    nc = _get_nc()
    in_map = {
        "x": np.ascontiguousarray(x, dtype=np.float32),
        "matrices": np.ascontiguousarray(matrices, dtype=np.float32),
    }
    in_maps = [in_map for _ in range(N_CORES)]
    return run_bass_kernel_spmd(nc, in_maps, list(range(N_CORES)), **kwargs)


def kernel(x, matrices):
    res = _run(x, matrices)
    return np.asarray(res.results[0]["out"], dtype=np.float32).reshape(4)

